# revision 13
# baseline (speedup 1.0000x reference)
"""Trainium2 Bass kernel for nn_DistributionLoss_6940667150680 (segment_reduce).

Math: with per-class sums S_c = sum_{i: Y_i=c} w_i and counts n_c,
    L2 = sum_i ||w_i - S_{Y_i}/n_{Y_i}||^2 = sum_i ||w_i||^2 - sum_c ||S_c||^2/n_c
so a single streaming pass over w1 suffices.

Sharding strategy (segment-key sharding): the host routes rows by class --
rows are stably sorted by label and each class is padded with zero rows to a
multiple of 128 so that every 128-row tile belongs to exactly one class.  The
padded tile stream is split evenly across the 8 cores.  Each core then only
needs per-TILE column sums (S_c = sum of its tiles' sums, reduced on host),
which turns the segment reduction into a dense streaming reduce.

v2 layout (per 64-tile chunk, CF = 64*128 = 8192 columns, fp8):
  - DMA: one 1 MB HBM->SBUF transfer per chunk (8 KB per-partition rows).
  - PE: 32 DoubleRow fp8 matmuls, one per TILE PAIR: lhsT = [128, 2, 64]
    selector masks (member t of pair p selects psum row 2p+t), rhs =
    [128, 2, 128] pair slab -> accumulate [64, 128] tile column sums in one
    psum bank.  DoubleRow contracts both tiles of the pair in one pass.
  - PE (gram): the last GRAM_PAIRS pairs' squares are computed on the PE:
    lhsT = rhs = [128, 2, 64] half-pair slab -> [64, 64] Gram accumulated in
    the *same* psum bank (cols 128..191); host reads only its diagonal
    (= per-feature sums of squares).  start=False rides the selector
    group's bank zero-fill.
  - ACT: Square activation with accum_out on the first 256*NA columns.
  - DVE: fused square+reduce (scalar_tensor_tensor) on the next 256*NV cols.
  - Evac: DVE tensor_copy moves the [64, 192] psum bank (tile sums + gram)
    to an SBUF bounce ring two chunks late (so DVE never waits on PE), and
    an SP-ring DMA ships it to HBM one chunk later still.  The psum WAR for
    bank reuse rides an SP nop ahead of each w DMA (mm[0]'s wait on the w
    DMA sem then implies the copy finished).
  - Host: fp8 cast + class-sorted layout (input prep), per-class reduction
    of tile sums, counts via bincount, final scalar in float64.
"""

import ml_dtypes
import numpy as np
from contextlib import ExitStack

import concourse.bass as bass
import concourse.tile as tile
from concourse import mybir
from concourse.bass_utils import run_bass_kernel_spmd

N_CORES = 8
D = 128           # feature dim
P = 128           # partitions / rows per tile
CHUNK = 64        # tiles per DMA chunk
PAIRS = CHUNK // 2
RING = 5          # w-ring depth in chunks
NPS = 4           # psum banks (round-robin per chunk)
CP_LAG = 2        # psum->SBUF copy issued this many chunks late (DVE no-stall)
EV_LAG = 3        # SBUF->HBM evac DMA issued this many chunks late
EB = 2            # evac SBUF bounce ring depth
NA = 13           # pairs squared on ACT  (256*NA columns)
NG = 9            # pairs squared on PE via Gram
# remaining PAIRS - NA - NG pairs squared on DVE
USE_DR = True     # DoubleRow fp8 matmuls (2 tiles per matmul)
NP_DT = ml_dtypes.float8_e4m3
EV = D + 64       # evac width per chunk: 128 tile-sum cols + 64 gram cols


def build_program(T: int, na: int = NA, ng: int = NG):
    """Per-core program processing T tiles (T % CHUNK == 0)."""
    f32, f16 = mybir.dt.float32, mybir.dt.float16
    fdt = mybir.dt.float8e4
    assert T % CHUNK == 0
    nch = T // CHUNK
    CF = CHUNK * D            # columns per chunk
    nv = PAIRS - na - ng      # DVE pairs
    assert nv >= 0
    fdA, fdV = 256 * na, 256 * nv
    DR = mybir.MatmulPerfMode.DoubleRow if USE_DR else None

    nc = bass.Bass()
    w_in = nc.dram_tensor("w", [P, T * D], fdt, kind="ExternalInput")
    masks_in = nc.dram_tensor("masks", [P, PAIRS * 2 * CHUNK], fdt, kind="ExternalInput")
    ts_out = nc.dram_tensor("ts_out", [CHUNK, nch * EV], f32, kind="ExternalOutput")
    sqa_out = nc.dram_tensor("sqa_out", [P, max(nch, 1)], f32, kind="ExternalOutput")
    sqv_out = nc.dram_tensor("sqv_out", [P, max(nch, 1)], f32, kind="ExternalOutput")

    def dep(frm, to, why):
        tile.add_dep_helper(
            getattr(frm, "ins", frm), getattr(to, "ins", to), reason=why
        )

    def demote(inst, dep_insts):
        """Move provably-redundant sync deps to nosync (ordering only):
        same-engine WAW/WAR (in-order engines) and deps transitively covered
        by another emitted wait (ISA structs hold one sync wait each)."""
        inst = getattr(inst, "ins", inst)
        drop = set()
        for d in dep_insts:
            if d is None:
                continue
            drop.add(getattr(d, "ins", d).name)
        syncs = inst.take_sync_dependencies()
        nosyncs = inst.take_nosync_dependencies()
        for name in drop & set(syncs):
            syncs.discard(name)
            nosyncs.add(name)
        inst.set_sync_dependencies(syncs)
        inst.set_nosync_dependencies(nosyncs)

    # Pin each engine queue to emission order with demoted (nosync) chain
    # edges: the tile scheduler may otherwise reorder within a queue, which
    # breaks every "covered transitively via in-order engine" argument below.
    last_on = {}

    def chain(inst, engine):
        prev = last_on.get(engine)
        if prev is not None:
            dep(inst, prev, "queue order")
            demote(inst, [prev])
        last_on[engine] = inst
        return inst

    with tile.TileContext(nc) as tc, ExitStack() as ctx:
        const = ctx.enter_context(tc.tile_pool(name="const", bufs=1))
        psum = ctx.enter_context(tc.tile_pool(name="psum", bufs=1, space="PSUM"))

        masks_sb = const.tile([P, PAIRS, 2, CHUNK], fdt, name="masks_sb")
        w_ring = const.tile([P, RING, PAIRS, 2, D], fdt, name="w_ring")
        sqa_cols = const.tile([P, max(nch, 1)], f32, name="sqa_cols")
        sqv_cols = const.tile([P, max(nch, 1)], f32, name="sqv_cols")
        scrA = const.tile([P, 2, max(fdA, 2)], f16, name="scrA")
        scrV = const.tile([P, 2, max(fdV, 2)], f16, name="scrV")
        evac_sb = const.tile([CHUNK, EB, EV], f32, name="evac_sb")

        # [64, 128] selector sums + [64, 64] gram share one psum bank each.
        pst = [psum.tile([CHUNK, EV], f32, name=f"pst{k}") for k in range(NPS)]

        dma_masks = chain(nc.sync.dma_start(out=masks_sb, in_=masks_in[:, :]), "sp")

        dmas = {}
        readers = {}   # chunk -> instructions that read its ring slot
        acts = {}
        ttrs = {}
        pe_last = {}   # chunk -> last PE instruction of the chunk
        sel_last = {}
        cps = {}       # chunk -> DVE psum->SBUF copy
        evacs = {}     # chunk -> SBUF->HBM evac DMA

        def emit_cp(c):
            """DVE: psum bank of chunk c -> SBUF bounce slot c%EB.  A DVE nop
            carries the bounce-slot WAR (evac DMA of chunk c-EB drained); the
            copy itself carries the PE-done wait."""
            if c - EB in evacs:
                nd = chain(nc.vector.engine_nop(), "dve")
                dep(nd, evacs[c - EB], "bounce slot free")
            cp = chain(
                nc.vector.tensor_copy(evac_sb[:, c % EB, :], pst[c % NPS][:, :]),
                "dve",
            )
            dep(cp, pe_last[c], "psum ready")
            demote(cp, [r for r in readers[c] if r is not pe_last[c]]
                   + list(cps.values()) + list(evacs.values()))
            cps[c] = cp

        def emit_evac(c):
            """SP: bounce slot of chunk c -> HBM, behind a nop carrying the
            copy-done wait."""
            spn = chain(nc.sync.nop(nofuse=True, hint=f"ev{c}"), "sp")
            dep(spn, cps[c], "copy done")
            ev = chain(
                nc.sync.dma_start(
                    out=ts_out[:, c * EV : (c + 1) * EV],
                    in_=evac_sb[:, c % EB, :],
                ),
                "sp",
            )
            dep(ev, spn, "after producer nop")
            demote(ev, [spn, cps[c]] + list(dmas.values()) + list(evacs.values()))
            evacs[c] = ev

        for c in range(nch):
            j = c % RING
            # Ring-slot WAR: carry waits on SP nops (the chain edges keep the
            # DMA behind them in the SP queue, so their hardware waits also
            # protect it).
            if c >= RING:
                n1 = chain(nc.sync.nop(nofuse=True, hint=f"war{c}a"), "sp")
                dep(n1, acts[c - RING], "act reader done")
                if (c - RING) in ttrs:
                    n1b = chain(nc.sync.nop(nofuse=True, hint=f"war{c}c"), "sp")
                    dep(n1b, ttrs[c - RING], "ttr reader done")
                n2 = chain(nc.sync.nop(nofuse=True, hint=f"war{c}b"), "sp")
                dep(n2, pe_last[c - RING], "pe reader done")
            if c - NPS in cps:
                # psum-bank WAR: gate this w DMA on the copy of chunk c-NPS;
                # mm[0]'s wait on the DMA sem then implies the bank is free.
                n3 = chain(nc.sync.nop(nofuse=True, hint=f"war{c}p"), "sp")
                dep(n3, cps[c - NPS], "psum bank free")
            dma = chain(
                nc.sync.dma_start(
                    out=w_ring[:, j, :, :, :], in_=w_in[:, c * CF : (c + 1) * CF]
                ),
                "sp",
            )
            # WAR waits live on the nops just above; DMA-vs-DMA WAW is ordered
            # by the HWDGE ring (FIFO per issuing engine).
            demote(dma, list(dmas.values()) + list(evacs.values()) + [dma_masks]
                   + [r for k in readers for r in readers[k]])
            dmas[c] = dma
            readers[c] = []

            # PE: selector sums.  One DoubleRow matmul per tile pair; member t
            # of pair p lands in psum row 2p+t.  mm[0] waits on the w DMA sem
            # directly; that also covers the masks DMA and the psum evac of
            # chunk c-NPS via the SP ring FIFO.
            pt = pst[c % NPS]
            # Deps provably covered elsewhere for every PE instruction of this
            # chunk: psum WAW vs chunk c-NPS rides the in-order PE queue; the
            # psum WAR vs evac_{c-NPS} rides the SP ring FIFO (mm[0]'s wait on
            # this chunk's w DMA implies the evac drained); the masks DMA
            # precedes every w DMA on the same ring.
            pe_covered = [dma_masks, cps.get(c - NPS), pe_last.get(c - 1),
                          pe_last.get(c - NPS), sel_last.get(c - NPS)]
            for p in range(PAIRS):
                mm = chain(
                    nc.tensor.matmul(
                        pt[:, 0:D],
                        lhsT=masks_sb[:, p, :, :],
                        rhs=w_ring[:, j, p, :, :],
                        start=(p == 0),
                        stop=(p == PAIRS - 1),
                        perf_mode=DR,
                    ),
                    "pe",
                )
                if p == 0:
                    demote(mm, pe_covered)
                else:
                    demote(mm, [dma] + pe_covered)
                readers[c].append(mm)
            sel_last[c] = mm
            # PE: gram squares for the last ng pairs (two half-feature
            # matmuls per pair into psum cols 128..191; start=False rides the
            # selector group's bank zero-fill).
            for gi in range(ng):
                p = PAIRS - ng + gi
                for h in range(2):
                    gm = chain(
                        nc.tensor.matmul(
                            pt[:, D : D + 64],
                            lhsT=w_ring[:, j, p, :, h * 64 : (h + 1) * 64],
                            rhs=w_ring[:, j, p, :, h * 64 : (h + 1) * 64],
                            start=False,
                            stop=(gi == ng - 1 and h == 1),
                            perf_mode=DR,
                            skip_group_check=True,
                        ),
                        "pe",
                    )
                    demote(gm, [dma, sel_last[c]] + pe_covered)
                    readers[c].append(gm)
            pe_last[c] = readers[c][-1]

            # ACT: squares of the first fdA columns, accumulated per chunk.
            act = chain(
                nc.scalar.activation(
                    scrA[:, c % 2, :],
                    w_ring[:, j, 0:na, :, :],
                    mybir.ActivationFunctionType.Square,
                    accum_out=sqa_cols[:, c : c + 1],
                ),
                "act",
            )
            demote(act, list(acts.values()))  # same-engine WAW on scrA
            acts[c] = act
            readers[c].append(act)

            # DVE: fused square+reduce of the middle fdV columns.
            if nv > 0:
                ttr = chain(
                    nc.vector.scalar_tensor_tensor(
                        out=scrV[:, c % 2, :],
                        in0=w_ring[:, j, na : na + nv, :, :],
                        scalar=1.0,
                        in1=w_ring[:, j, na : na + nv, :, :],
                        op0=mybir.AluOpType.mult,
                        op1=mybir.AluOpType.mult,
                        accum_out=sqv_cols[:, c : c + 1],
                    ),
                    "dve",
                )
                demote(ttr, list(ttrs.values()))  # scrV WAW same-engine
                ttrs[c] = ttr
                readers[c].append(ttr)

            # Lagged psum evac: by now PE finished chunk c-CP_LAG long ago,
            # so neither DVE nor SP stalls and the w-DMA prefetch stays deep.
            if c >= CP_LAG:
                emit_cp(c - CP_LAG)
            if c >= EV_LAG:
                emit_evac(c - EV_LAG)

        # Tail flush, same cadence as the main loop so bounce-slot reuse
        # (cp of chunk c vs evac DMA of chunk c-EB) stays correctly ordered.
        for cc in range(nch, nch + EV_LAG):
            if 0 <= cc - CP_LAG < nch and (cc - CP_LAG) not in cps:
                emit_cp(cc - CP_LAG)
            if 0 <= cc - EV_LAG < nch and (cc - EV_LAG) not in evacs:
                emit_evac(cc - EV_LAG)

        # Outputs: each DMA waits on the last producer via an SP nop.
        outs = []
        for name, buf, src, last in (
            ("sqa", sqa_out, sqa_cols, acts[nch - 1]),
            ("sqv", sqv_out, sqv_cols, ttrs.get(nch - 1)),
        ):
            if last is None:
                continue
            spn = chain(nc.sync.nop(nofuse=True, hint=f"out_{name}"), "sp")
            dep(spn, last, f"{name} ready")
            od = chain(nc.sync.dma_start(out=buf[:, :], in_=src), "sp")
            dep(od, spn, "after producer nop")
            demote(od, [spn, last] + list(dmas.values()) + outs
                   + list(evacs.values()) + list(acts.values())
                   + list(ttrs.values()))
            outs.append(od)

        # Tail sync: cover every proc with single-wait SP nops.
        tails = [pe_last[nch - 1], acts[nch - 1]] + outs + [evacs[nch - 1]]
        if (nch - 1) in ttrs:
            tails.append(ttrs[nch - 1])
        for t in tails:
            nop = chain(nc.sync.nop(nofuse=True, hint="tailcover"), "sp")
            dep(nop, t, "tail")

    # The kernel-tail drain waits on every proc; its NOP struct cannot hold
    # that many sync waits and the SP-queue nops above already cover them.
    for blk in nc.m.functions[0].blocks:
        for inst in blk.instructions:
            if not isinstance(inst, mybir.InstDrain):
                continue
            si = inst.sync_info
            if si is None or len(si.on_wait) <= 2:
                continue
            inst.sync_info = mybir.SyncInfo(on_wait=[], on_update=list(si.on_update))

    return nc


def prepare_inputs(w1: np.ndarray, Y: np.ndarray, num_classes: int):
    """Class-sorted, per-class tile-padded, per-core partition-major fp8."""
    n = w1.shape[0]
    counts = np.bincount(Y, minlength=num_classes).astype(np.int64)
    tpc_class = (counts + P - 1) // P          # tiles per class
    pad_start = np.zeros(num_classes + 1, dtype=np.int64)
    np.cumsum(tpc_class, out=pad_start[1:])
    tt = int(pad_start[-1])                    # total real tiles
    t_core = -(-tt // N_CORES)                 # ceil
    t_core = -(-t_core // CHUNK) * CHUNK       # round up to chunk
    t_total = t_core * N_CORES

    order = np.argsort(Y, kind="stable")
    y_sorted = Y[order]
    class_start = np.zeros(num_classes, dtype=np.int64)
    class_start[1:] = np.cumsum(counts)[:-1]
    rank = np.arange(n, dtype=np.int64) - class_start[y_sorted]
    dest = pad_start[y_sorted] * P + rank

    w16 = np.zeros((t_total * P, D), dtype=NP_DT)
    w16[dest] = w1[order].astype(NP_DT)

    # selector masks: masks[k, p, t, m] = (m == 2p+t), laid out
    # [P, PAIRS*2*CHUNK]
    m1 = np.zeros((PAIRS, 2, CHUNK), dtype=NP_DT)
    for p in range(PAIRS):
        m1[p, 0, 2 * p] = 1
        m1[p, 1, 2 * p + 1] = 1
    masks = np.ascontiguousarray(
        np.broadcast_to(m1.reshape(1, PAIRS * 2 * CHUNK), (P, PAIRS * 2 * CHUNK))
    )
    in_maps = []
    for k in range(N_CORES):
        blk = w16[k * t_core * P : (k + 1) * t_core * P]
        wk = np.ascontiguousarray(
            blk.reshape(t_core, P, D).transpose(1, 0, 2).reshape(P, t_core * D)
        )
        in_maps.append({"w": wk, "masks": masks})
    return in_maps, t_core, pad_start, counts


def combine(results, t_core, pad_start, counts, n_total):
    """Host-side: tile sums -> class sums -> final scalar, in float64."""
    nch = t_core // CHUNK
    blocks = [r["ts_out"].astype(np.float64).reshape(CHUNK, nch, EV) for r in results]
    tile_sums = np.concatenate(
        [b[:, :, :D].transpose(1, 0, 2).reshape(t_core, D) for b in blocks],
        axis=0,
    )  # [t_total, D]
    num_classes = len(counts)
    totsq = 0.0
    idx = np.arange(CHUNK)
    for r, b in zip(results, blocks):
        totsq += float(r["sqa_out"].astype(np.float64).sum())
        totsq += float(r["sqv_out"].astype(np.float64).sum())
        # gram diagonals: b[:, :, D:] is [64, nch, 64]; diag over (row, col)
        gram = b[:64, :, D : D + 64]
        totsq += float(gram[idx[:64], :, idx[:64]].sum())
    # per-class sums: classes are tile-aligned runs of tile_sums
    seg = np.add.reduceat(tile_sums[: pad_start[-1]], pad_start[:-1], axis=0) \
        if pad_start[-1] > 0 else np.zeros((num_classes, D))
    # reduceat quirk: empty segments (pad_start[c]==pad_start[c+1]) copy the
    # row at that index instead of 0 -- mask them out via counts.
    nz = counts > 0
    s = seg[nz]
    corr = float(((s * s).sum(axis=1) / counts[nz]).sum())
    return np.float32((totsq - corr) / n_total)


def run_sharded(w1: np.ndarray, Y: np.ndarray, num_classes: int, trace: bool = False):
    w1 = np.ascontiguousarray(np.asarray(w1, dtype=np.float32))
    Y = np.asarray(Y).astype(np.int64)
    in_maps, t_core, pad_start, counts = prepare_inputs(w1, Y, num_classes)
    nc = build_program(t_core)
    out = run_bass_kernel_spmd(nc, in_maps, list(range(N_CORES)), trace=trace)
    value = combine(out.results, t_core, pad_start, counts, w1.shape[0])
    return value, out


def kernel(w1, Y, num_classes=None):
    w1 = np.asarray(w1, dtype=np.float32)
    Y = np.asarray(Y)
    c = int(np.asarray(num_classes)) if num_classes is not None else 1000
    assert w1.ndim == 2 and w1.shape[1] == D
    value, _ = run_sharded(w1, Y, c, trace=False)
    return value


# revision 16
# speedup vs baseline: 1.4125x; 1.4125x over previous
"""Trainium2 Bass kernel for nn_DistributionLoss_6940667150680 (segment_reduce).

Math: with per-class sums S_c = sum_{i: Y_i=c} w_i and counts n_c,
    L2 = sum_i ||w_i - S_{Y_i}/n_{Y_i}||^2 = sum_i ||w_i||^2 - sum_c ||S_c||^2/n_c
so a single streaming pass over w1 suffices.

Sharding strategy (segment-key sharding): the host routes rows by class --
rows are stably sorted by label and each class is padded with zero rows to a
multiple of 128 so that every 128-row tile belongs to exactly one class.  The
padded tile stream is split evenly across the 8 cores.  Each core then only
needs per-TILE column sums (S_c = sum of its tiles' sums, reduced on host),
which turns the segment reduction into a dense streaming reduce.

v2 layout (per 64-tile chunk, CF = 64*128 = 8192 columns, fp8):
  - DMA: one 1 MB HBM->SBUF transfer per chunk (8 KB per-partition rows).
  - PE: 32 DoubleRow fp8 matmuls, one per TILE PAIR: lhsT = [128, 2, 64]
    selector masks (member t of pair p selects psum row 2p+t), rhs =
    [128, 2, 128] pair slab -> accumulate [64, 128] tile column sums in one
    psum bank.  DoubleRow contracts both tiles of the pair in one pass.
  - PE (gram): the last GRAM_PAIRS pairs' squares are computed on the PE:
    lhsT = rhs = [128, 2, 64] half-pair slab -> [64, 64] Gram accumulated in
    the *same* psum bank (cols 128..191); host reads only its diagonal
    (= per-feature sums of squares).  start=False rides the selector
    group's bank zero-fill.
  - ACT: Square activation with accum_out on the first 256*NA columns.
  - DVE: fused square+reduce (scalar_tensor_tensor) on the next 256*NV cols.
  - Evac: DVE tensor_copy moves the [64, 192] psum bank (tile sums + gram)
    to an SBUF bounce ring two chunks late (so DVE never waits on PE), and
    an SP-ring DMA ships it to HBM one chunk later still.  The psum WAR for
    bank reuse rides an SP nop ahead of each w DMA (mm[0]'s wait on the w
    DMA sem then implies the copy finished).
  - Host: fp8 cast + class-sorted layout (input prep), per-class reduction
    of tile sums, counts via bincount, final scalar in float64.
"""

import ml_dtypes
import numpy as np
from contextlib import ExitStack

import concourse.bass as bass
import concourse.tile as tile
from concourse import mybir
from concourse.bass_utils import run_bass_kernel_spmd

N_CORES = 8
D = 128           # feature dim
P = 128           # partitions / rows per tile
CHUNK = 64        # tiles per DMA chunk
PAIRS = CHUNK // 2
RING = 6          # w-ring depth in chunks
NPS = 6           # psum banks (round-robin per chunk)
CP_LAG = 2        # psum->SBUF copy issued this many chunks late (DVE no-stall)
EV_LAG = 5        # SBUF->HBM evac DMA issued this many chunks late
EB = 6            # evac SBUF bounce ring depth
NA = 13           # pairs squared on ACT  (256*NA columns)
NG = 9            # pairs squared on PE via Gram
# remaining PAIRS - NA - NG pairs squared on DVE
USE_DR = True     # DoubleRow fp8 matmuls (2 tiles per matmul)
NP_DT = ml_dtypes.float8_e4m3
EV = D + 64       # evac width per chunk: 128 tile-sum cols + 64 gram cols


def build_program(T: int, na: int = NA, ng: int = NG):
    """Per-core program processing T tiles (T % CHUNK == 0)."""
    f32, f16 = mybir.dt.float32, mybir.dt.float16
    fdt = mybir.dt.float8e4
    assert T % CHUNK == 0
    nch = T // CHUNK
    CF = CHUNK * D            # columns per chunk
    nv = PAIRS - na - ng      # DVE pairs
    assert nv >= 0
    fdA, fdV = 256 * na, 256 * nv
    DR = mybir.MatmulPerfMode.DoubleRow if USE_DR else None

    nc = bass.Bass()
    w_in = nc.dram_tensor("w", [P, T * D], fdt, kind="ExternalInput")
    masks_in = nc.dram_tensor("masks", [P, PAIRS * 2 * CHUNK], fdt, kind="ExternalInput")
    ts_out = nc.dram_tensor("ts_out", [CHUNK, nch * EV], f32, kind="ExternalOutput")
    sqa_out = nc.dram_tensor("sqa_out", [P, max(nch, 1)], f32, kind="ExternalOutput")
    sqv_out = nc.dram_tensor("sqv_out", [P, max(nch, 1)], f32, kind="ExternalOutput")

    def dep(frm, to, why):
        tile.add_dep_helper(
            getattr(frm, "ins", frm), getattr(to, "ins", to), reason=why
        )

    def demote(inst, dep_insts):
        """Move provably-redundant sync deps to nosync (ordering only):
        same-engine WAW/WAR (in-order engines) and deps transitively covered
        by another emitted wait (ISA structs hold one sync wait each)."""
        inst = getattr(inst, "ins", inst)
        drop = set()
        for d in dep_insts:
            if d is None:
                continue
            drop.add(getattr(d, "ins", d).name)
        syncs = inst.take_sync_dependencies()
        nosyncs = inst.take_nosync_dependencies()
        for name in drop & set(syncs):
            syncs.discard(name)
            nosyncs.add(name)
        inst.set_sync_dependencies(syncs)
        inst.set_nosync_dependencies(nosyncs)

    # Pin each engine queue to emission order with demoted (nosync) chain
    # edges: the tile scheduler may otherwise reorder within a queue, which
    # breaks every "covered transitively via in-order engine" argument below.
    last_on = {}

    def chain(inst, engine):
        prev = last_on.get(engine)
        if prev is not None:
            dep(inst, prev, "queue order")
            demote(inst, [prev])
        last_on[engine] = inst
        return inst

    with tile.TileContext(nc) as tc, ExitStack() as ctx:
        const = ctx.enter_context(tc.tile_pool(name="const", bufs=1))
        psum = ctx.enter_context(tc.tile_pool(name="psum", bufs=1, space="PSUM"))

        masks_sb = const.tile([P, PAIRS, 2, CHUNK], fdt, name="masks_sb")
        w_ring = const.tile([P, RING, PAIRS, 2, D], fdt, name="w_ring")
        sqa_cols = const.tile([P, max(nch, 1)], f32, name="sqa_cols")
        sqv_cols = const.tile([P, max(nch, 1)], f32, name="sqv_cols")
        scrA = const.tile([P, 2, max(fdA, 2)], f16, name="scrA")
        scrV = const.tile([P, 2, max(fdV, 2)], f16, name="scrV")
        evac_sb = const.tile([CHUNK, EB, EV], f32, name="evac_sb")

        # [64, 128] selector sums + [64, 64] gram share one psum bank each.
        pst = [psum.tile([CHUNK, EV], f32, name=f"pst{k}") for k in range(NPS)]

        dma_masks = chain(nc.sync.dma_start(out=masks_sb, in_=masks_in[:, :]), "sp")

        dmas = {}
        readers = {}   # chunk -> instructions that read its ring slot
        acts = {}
        ttrs = {}
        pe_last = {}   # chunk -> last PE instruction of the chunk
        sel_last = {}
        cps = {}       # chunk -> DVE psum->SBUF copy
        evacs = {}     # chunk -> SBUF->HBM evac DMA

        def emit_cp(c):
            """DVE: psum bank of chunk c -> SBUF bounce slot c%EB.  A DVE nop
            carries the bounce-slot WAR (evac DMA of chunk c-EB drained); the
            copy itself carries the PE-done wait."""
            if c - EB in evacs:
                nd = chain(nc.vector.engine_nop(), "dve")
                dep(nd, evacs[c - EB], "bounce slot free")
            cp = chain(
                nc.vector.tensor_copy(evac_sb[:, c % EB, :], pst[c % NPS][:, :]),
                "dve",
            )
            dep(cp, pe_last[c], "psum ready")
            demote(cp, [r for r in readers[c] if r is not pe_last[c]]
                   + list(cps.values()) + list(evacs.values()))
            cps[c] = cp

        def emit_evac(c):
            """SP: bounce slot of chunk c -> HBM, behind a nop carrying the
            copy-done wait."""
            spn = chain(nc.sync.nop(nofuse=True, hint=f"ev{c}"), "sp")
            dep(spn, cps[c], "copy done")
            ev = chain(
                nc.sync.dma_start(
                    out=ts_out[:, c * EV : (c + 1) * EV],
                    in_=evac_sb[:, c % EB, :],
                ),
                "sp",
            )
            dep(ev, spn, "after producer nop")
            demote(ev, [spn, cps[c]] + list(dmas.values()) + list(evacs.values()))
            evacs[c] = ev

        for c in range(nch):
            j = c % RING
            # Ring-slot WAR: carry waits on SP nops (the chain edges keep the
            # DMA behind them in the SP queue, so their hardware waits also
            # protect it).
            if c >= RING:
                n1 = chain(nc.sync.nop(nofuse=True, hint=f"war{c}a"), "sp")
                dep(n1, acts[c - RING], "act reader done")
                if (c - RING) in ttrs:
                    n1b = chain(nc.sync.nop(nofuse=True, hint=f"war{c}c"), "sp")
                    dep(n1b, ttrs[c - RING], "ttr reader done")
                n2 = chain(nc.sync.nop(nofuse=True, hint=f"war{c}b"), "sp")
                dep(n2, pe_last[c - RING], "pe reader done")
            if c - NPS in cps:
                # psum-bank WAR: gate this w DMA on the copy of chunk c-NPS;
                # mm[0]'s wait on the DMA sem then implies the bank is free.
                n3 = chain(nc.sync.nop(nofuse=True, hint=f"war{c}p"), "sp")
                dep(n3, cps[c - NPS], "psum bank free")
            dma = chain(
                nc.sync.dma_start(
                    out=w_ring[:, j, :, :, :], in_=w_in[:, c * CF : (c + 1) * CF]
                ),
                "sp",
            )
            # WAR waits live on the nops just above; DMA-vs-DMA WAW is ordered
            # by the HWDGE ring (FIFO per issuing engine).
            demote(dma, list(dmas.values()) + list(evacs.values()) + [dma_masks]
                   + [r for k in readers for r in readers[k]])
            dmas[c] = dma
            readers[c] = []

            # PE: selector sums.  One DoubleRow matmul per tile pair; member t
            # of pair p lands in psum row 2p+t.  mm[0] waits on the w DMA sem
            # directly; that also covers the masks DMA and the psum evac of
            # chunk c-NPS via the SP ring FIFO.
            pt = pst[c % NPS]
            # Deps provably covered elsewhere for every PE instruction of this
            # chunk: psum WAW vs chunk c-NPS rides the in-order PE queue; the
            # psum WAR vs evac_{c-NPS} rides the SP ring FIFO (mm[0]'s wait on
            # this chunk's w DMA implies the evac drained); the masks DMA
            # precedes every w DMA on the same ring.
            pe_covered = [dma_masks, cps.get(c - NPS), pe_last.get(c - 1),
                          pe_last.get(c - NPS), sel_last.get(c - NPS)]
            for p in range(PAIRS):
                mm = chain(
                    nc.tensor.matmul(
                        pt[:, 0:D],
                        lhsT=masks_sb[:, p, :, :],
                        rhs=w_ring[:, j, p, :, :],
                        start=(p == 0),
                        stop=(p == PAIRS - 1),
                        perf_mode=DR,
                    ),
                    "pe",
                )
                if p == 0:
                    demote(mm, pe_covered)
                else:
                    demote(mm, [dma] + pe_covered)
                readers[c].append(mm)
            sel_last[c] = mm
            # PE: gram squares for the last ng pairs (two half-feature
            # matmuls per pair into psum cols 128..191; start=False rides the
            # selector group's bank zero-fill).
            for gi in range(ng):
                p = PAIRS - ng + gi
                for h in range(2):
                    gm = chain(
                        nc.tensor.matmul(
                            pt[:, D : D + 64],
                            lhsT=w_ring[:, j, p, :, h * 64 : (h + 1) * 64],
                            rhs=w_ring[:, j, p, :, h * 64 : (h + 1) * 64],
                            start=False,
                            stop=(gi == ng - 1 and h == 1),
                            perf_mode=DR,
                            skip_group_check=True,
                        ),
                        "pe",
                    )
                    demote(gm, [dma, sel_last[c]] + pe_covered)
                    readers[c].append(gm)
            pe_last[c] = readers[c][-1]

            # ACT: squares of the first fdA columns, accumulated per chunk.
            act = chain(
                nc.scalar.activation(
                    scrA[:, c % 2, :],
                    w_ring[:, j, 0:na, :, :],
                    mybir.ActivationFunctionType.Square,
                    accum_out=sqa_cols[:, c : c + 1],
                ),
                "act",
            )
            demote(act, list(acts.values()))  # same-engine WAW on scrA
            acts[c] = act
            readers[c].append(act)

            # Lagged psum->SBUF copy, emitted BEFORE this chunk's ttr so the
            # SP-side waits on it (psum WAR, evac) never chain behind a fresh
            # ttr in the in-order DVE queue.
            if c >= CP_LAG:
                emit_cp(c - CP_LAG)

            # DVE: fused square+reduce of the middle fdV columns.
            if nv > 0:
                ttr = chain(
                    nc.vector.scalar_tensor_tensor(
                        out=scrV[:, c % 2, :],
                        in0=w_ring[:, j, na : na + nv, :, :],
                        scalar=1.0,
                        in1=w_ring[:, j, na : na + nv, :, :],
                        op0=mybir.AluOpType.mult,
                        op1=mybir.AluOpType.mult,
                        accum_out=sqv_cols[:, c : c + 1],
                    ),
                    "dve",
                )
                demote(ttr, list(ttrs.values()))  # scrV WAW same-engine
                ttrs[c] = ttr
                readers[c].append(ttr)

            # Lagged SBUF->HBM evac: by now the copy of chunk c-EV_LAG is long
            # done, so the SP queue doesn't stall and the prefetch stays deep.
            if c >= EV_LAG:
                emit_evac(c - EV_LAG)

        # Tail flush, same cadence as the main loop so bounce-slot reuse
        # (cp of chunk c vs evac DMA of chunk c-EB) stays correctly ordered.
        for cc in range(nch, nch + EV_LAG):
            if 0 <= cc - CP_LAG < nch and (cc - CP_LAG) not in cps:
                emit_cp(cc - CP_LAG)
            if 0 <= cc - EV_LAG < nch and (cc - EV_LAG) not in evacs:
                emit_evac(cc - EV_LAG)

        # Outputs: each DMA waits on the last producer via an SP nop.
        outs = []
        for name, buf, src, last in (
            ("sqa", sqa_out, sqa_cols, acts[nch - 1]),
            ("sqv", sqv_out, sqv_cols, ttrs.get(nch - 1)),
        ):
            if last is None:
                continue
            spn = chain(nc.sync.nop(nofuse=True, hint=f"out_{name}"), "sp")
            dep(spn, last, f"{name} ready")
            od = chain(nc.sync.dma_start(out=buf[:, :], in_=src), "sp")
            dep(od, spn, "after producer nop")
            demote(od, [spn, last] + list(dmas.values()) + outs
                   + list(evacs.values()) + list(acts.values())
                   + list(ttrs.values()))
            outs.append(od)

        # Tail sync: cover every proc with single-wait SP nops.
        tails = [pe_last[nch - 1], acts[nch - 1]] + outs + [evacs[nch - 1]]
        if (nch - 1) in ttrs:
            tails.append(ttrs[nch - 1])
        for t in tails:
            nop = chain(nc.sync.nop(nofuse=True, hint="tailcover"), "sp")
            dep(nop, t, "tail")

    # The kernel-tail drain waits on every proc; its NOP struct cannot hold
    # that many sync waits and the SP-queue nops above already cover them.
    for blk in nc.m.functions[0].blocks:
        for inst in blk.instructions:
            if not isinstance(inst, mybir.InstDrain):
                continue
            si = inst.sync_info
            if si is None or len(si.on_wait) <= 2:
                continue
            inst.sync_info = mybir.SyncInfo(on_wait=[], on_update=list(si.on_update))

    return nc


def prepare_inputs(w1: np.ndarray, Y: np.ndarray, num_classes: int):
    """Class-sorted, per-class tile-padded, per-core partition-major fp8."""
    n = w1.shape[0]
    counts = np.bincount(Y, minlength=num_classes).astype(np.int64)
    tpc_class = (counts + P - 1) // P          # tiles per class
    pad_start = np.zeros(num_classes + 1, dtype=np.int64)
    np.cumsum(tpc_class, out=pad_start[1:])
    tt = int(pad_start[-1])                    # total real tiles
    t_core = -(-tt // N_CORES)                 # ceil
    t_core = -(-t_core // CHUNK) * CHUNK       # round up to chunk
    t_total = t_core * N_CORES

    order = np.argsort(Y, kind="stable")
    y_sorted = Y[order]
    class_start = np.zeros(num_classes, dtype=np.int64)
    class_start[1:] = np.cumsum(counts)[:-1]
    rank = np.arange(n, dtype=np.int64) - class_start[y_sorted]
    dest = pad_start[y_sorted] * P + rank

    w16 = np.zeros((t_total * P, D), dtype=NP_DT)
    w16[dest] = w1[order].astype(NP_DT)

    # selector masks: masks[k, p, t, m] = (m == 2p+t), laid out
    # [P, PAIRS*2*CHUNK]
    m1 = np.zeros((PAIRS, 2, CHUNK), dtype=NP_DT)
    for p in range(PAIRS):
        m1[p, 0, 2 * p] = 1
        m1[p, 1, 2 * p + 1] = 1
    masks = np.ascontiguousarray(
        np.broadcast_to(m1.reshape(1, PAIRS * 2 * CHUNK), (P, PAIRS * 2 * CHUNK))
    )
    in_maps = []
    for k in range(N_CORES):
        blk = w16[k * t_core * P : (k + 1) * t_core * P]
        wk = np.ascontiguousarray(
            blk.reshape(t_core, P, D).transpose(1, 0, 2).reshape(P, t_core * D)
        )
        in_maps.append({"w": wk, "masks": masks})
    return in_maps, t_core, pad_start, counts


def combine(results, t_core, pad_start, counts, n_total):
    """Host-side: tile sums -> class sums -> final scalar, in float64."""
    nch = t_core // CHUNK
    blocks = [r["ts_out"].astype(np.float64).reshape(CHUNK, nch, EV) for r in results]
    tile_sums = np.concatenate(
        [b[:, :, :D].transpose(1, 0, 2).reshape(t_core, D) for b in blocks],
        axis=0,
    )  # [t_total, D]
    num_classes = len(counts)
    totsq = 0.0
    idx = np.arange(CHUNK)
    for r, b in zip(results, blocks):
        totsq += float(r["sqa_out"].astype(np.float64).sum())
        totsq += float(r["sqv_out"].astype(np.float64).sum())
        # gram diagonals: b[:, :, D:] is [64, nch, 64]; diag over (row, col)
        gram = b[:64, :, D : D + 64]
        totsq += float(gram[idx[:64], :, idx[:64]].sum())
    # per-class sums: classes are tile-aligned runs of tile_sums
    seg = np.add.reduceat(tile_sums[: pad_start[-1]], pad_start[:-1], axis=0) \
        if pad_start[-1] > 0 else np.zeros((num_classes, D))
    # reduceat quirk: empty segments (pad_start[c]==pad_start[c+1]) copy the
    # row at that index instead of 0 -- mask them out via counts.
    nz = counts > 0
    s = seg[nz]
    corr = float(((s * s).sum(axis=1) / counts[nz]).sum())
    return np.float32((totsq - corr) / n_total)


def run_sharded(w1: np.ndarray, Y: np.ndarray, num_classes: int, trace: bool = False):
    w1 = np.ascontiguousarray(np.asarray(w1, dtype=np.float32))
    Y = np.asarray(Y).astype(np.int64)
    in_maps, t_core, pad_start, counts = prepare_inputs(w1, Y, num_classes)
    nc = build_program(t_core)
    out = run_bass_kernel_spmd(nc, in_maps, list(range(N_CORES)), trace=trace)
    value = combine(out.results, t_core, pad_start, counts, w1.shape[0])
    return value, out


def kernel(w1, Y, num_classes=None):
    w1 = np.asarray(w1, dtype=np.float32)
    Y = np.asarray(Y)
    c = int(np.asarray(num_classes)) if num_classes is not None else 1000
    assert w1.ndim == 2 and w1.shape[1] == D
    value, _ = run_sharded(w1, Y, c, trace=False)
    return value


# revision 23
# speedup vs baseline: 1.4479x; 1.0251x over previous
"""Trainium2 Bass kernel for nn_DistributionLoss_6940667150680 (segment_reduce).

Math: with per-class sums S_c = sum_{i: Y_i=c} w_i and counts n_c,
    L2 = sum_i ||w_i - S_{Y_i}/n_{Y_i}||^2 = sum_i ||w_i||^2 - sum_c ||S_c||^2/n_c
so a single streaming pass over w1 suffices.

Sharding strategy (segment-key sharding): the host routes rows by class --
rows are stably sorted by label and each class is padded with zero rows to a
multiple of 128 so that every 128-row tile belongs to exactly one class.  The
padded tile stream is split evenly across the 8 cores.  Each core then only
needs per-TILE column sums (S_c = sum of its tiles' sums, reduced on host),
which turns the segment reduction into a dense streaming reduce.

v2 layout (per 64-tile chunk, CF = 64*128 = 8192 columns, fp8):
  - DMA: one 1 MB HBM->SBUF transfer per chunk (8 KB per-partition rows).
  - PE: 32 DoubleRow fp8 matmuls, one per TILE PAIR: lhsT = [128, 2, 64]
    selector masks (member t of pair p selects psum row 2p+t), rhs =
    [128, 2, 128] pair slab -> accumulate [64, 128] tile column sums in one
    psum bank.  DoubleRow contracts both tiles of the pair in one pass.
  - PE (gram): the last GRAM_PAIRS pairs' squares are computed on the PE:
    lhsT = rhs = [128, 2, 64] half-pair slab -> [64, 64] Gram accumulated in
    the *same* psum bank (cols 128..191); host reads only its diagonal
    (= per-feature sums of squares).  start=False rides the selector
    group's bank zero-fill.
  - ACT: Square activation with accum_out on the first 256*NA columns.
  - DVE: fused square+reduce (scalar_tensor_tensor) on the next 256*NV cols.
  - Evac: DVE tensor_copy moves the [64, 192] psum bank (tile sums + gram)
    to an SBUF bounce ring two chunks late (so DVE never waits on PE), and
    an SP-ring DMA ships it to HBM one chunk later still.  The psum WAR for
    bank reuse rides an SP nop ahead of each w DMA (mm[0]'s wait on the w
    DMA sem then implies the copy finished).
  - Host: fp8 cast + class-sorted layout (input prep), per-class reduction
    of tile sums, counts via bincount, final scalar in float64.
"""

import ml_dtypes
import numpy as np
from contextlib import ExitStack

import concourse.bass as bass
import concourse.tile as tile
from concourse import mybir
from concourse.bass_utils import run_bass_kernel_spmd

N_CORES = 8
D = 128           # feature dim
P = 128           # partitions / rows per tile
CHUNK = 64        # tiles per DMA chunk
PAIRS = CHUNK // 2
RING = 6          # w-ring depth in chunks
NPS = 6           # psum banks (round-robin per chunk)
CP_LAG = 2        # psum->SBUF copy issued this many chunks late (DVE no-stall)
EV_LAG = 5        # SBUF->HBM evac DMA issued this many chunks late
EB = 6            # evac SBUF bounce ring depth
NA = 12           # pairs squared on ACT  (256*NA columns)
NG = 10           # pairs squared on PE via Gram
# remaining PAIRS - NA - NG pairs squared on DVE
USE_DR = True     # DoubleRow fp8 matmuls (2 tiles per matmul)
NP_DT = ml_dtypes.float8_e4m3
EV = D             # evac width per chunk: 128 tile-sum cols


def build_program(T: int, na: int = NA, ng: int = NG):
    """Per-core program processing T tiles (T % CHUNK == 0)."""
    f32, f16 = mybir.dt.float32, mybir.dt.float16
    fdt = mybir.dt.float8e4
    assert T % CHUNK == 0
    nch = T // CHUNK
    CF = CHUNK * D            # columns per chunk
    nv = PAIRS - na - ng      # DVE pairs
    assert nv >= 0
    fdA, fdV = 256 * na, 256 * nv
    DR = mybir.MatmulPerfMode.DoubleRow if USE_DR else None

    nc = bass.Bass()
    w_in = nc.dram_tensor("w", [P, T * D], fdt, kind="ExternalInput")
    masks_in = nc.dram_tensor("masks", [P, PAIRS * 2 * CHUNK], fdt, kind="ExternalInput")
    ts_out = nc.dram_tensor("ts_out", [CHUNK, nch * EV], f32, kind="ExternalOutput")
    sqa_out = nc.dram_tensor("sqa_out", [P, max(nch, 1)], f32, kind="ExternalOutput")
    sqv_out = nc.dram_tensor("sqv_out", [P, max(nch, 1)], f32, kind="ExternalOutput")
    gram_out = nc.dram_tensor("gram_out", [64, 64], f32, kind="ExternalOutput")

    def dep(frm, to, why):
        tile.add_dep_helper(
            getattr(frm, "ins", frm), getattr(to, "ins", to), reason=why
        )

    def demote(inst, dep_insts):
        """Move provably-redundant sync deps to nosync (ordering only):
        same-engine WAW/WAR (in-order engines) and deps transitively covered
        by another emitted wait (ISA structs hold one sync wait each)."""
        inst = getattr(inst, "ins", inst)
        drop = set()
        for d in dep_insts:
            if d is None:
                continue
            drop.add(getattr(d, "ins", d).name)
        syncs = inst.take_sync_dependencies()
        nosyncs = inst.take_nosync_dependencies()
        for name in drop & set(syncs):
            syncs.discard(name)
            nosyncs.add(name)
        inst.set_sync_dependencies(syncs)
        inst.set_nosync_dependencies(nosyncs)

    # Pin each engine queue to emission order with demoted (nosync) chain
    # edges: the tile scheduler may otherwise reorder within a queue, which
    # breaks every "covered transitively via in-order engine" argument below.
    last_on = {}

    def chain(inst, engine):
        prev = last_on.get(engine)
        if prev is not None:
            dep(inst, prev, "queue order")
            demote(inst, [prev])
        last_on[engine] = inst
        return inst

    with tile.TileContext(nc) as tc, ExitStack() as ctx:
        const = ctx.enter_context(tc.tile_pool(name="const", bufs=1))
        psum = ctx.enter_context(tc.tile_pool(name="psum", bufs=1, space="PSUM"))

        masks_sb = const.tile([P, PAIRS, 2, CHUNK], fdt, name="masks_sb")
        w_ring = const.tile([P, RING, PAIRS, 2, D], fdt, name="w_ring")
        sqa_cols = const.tile([P, max(nch, 1)], f32, name="sqa_cols")
        sqv_cols = const.tile([P, max(nch, 1)], f32, name="sqv_cols")
        scrA = const.tile([P, 2, max(fdA, 2)], f16, name="scrA")
        scrV = const.tile([P, 2, max(fdV, 2)], f16, name="scrV")
        evac_sb = const.tile([CHUNK, EB, EV], f32, name="evac_sb")
        gram_sb = const.tile([64, 64], f32, name="gram_sb")

        pst = [psum.tile([CHUNK, EV], f32, name=f"pst{k}") for k in range(NPS)]
        # One gram bank accumulates squares across the whole kernel; its
        # diagonal is read once at the end.
        gps = psum.tile([64, 64], f32, name="gps")

        # Masks ride the ACT HWDGE ring so they don't delay the first w chunk
        # on the SP ring (ACT is idle until the first chunk lands anyway).
        dma_masks = chain(nc.scalar.dma_start(out=masks_sb, in_=masks_in[:, :]),
                          "act")

        dmas = {}
        readers = {}   # chunk -> instructions that read its ring slot
        acts = {}
        ttrs = {}
        pe_last = {}   # chunk -> last PE instruction of the chunk
        sel_last = {}
        cps = {}       # chunk -> DVE psum->SBUF copy
        evacs = {}     # chunk -> SBUF->HBM evac DMA

        def emit_cp(c):
            """DVE: psum bank of chunk c -> SBUF bounce slot c%EB.  A DVE nop
            carries the bounce-slot WAR (evac DMA of chunk c-EB drained); the
            copy itself carries the PE-done wait."""
            if c - EB in evacs:
                nd = chain(nc.vector.engine_nop(), "dve")
                dep(nd, evacs[c - EB], "bounce slot free")
            cp = chain(
                nc.vector.tensor_copy(evac_sb[:, c % EB, :], pst[c % NPS][:, :]),
                "dve",
            )
            dep(cp, pe_last[c], "psum ready")
            demote(cp, [r for r in readers[c] if r is not pe_last[c]]
                   + list(cps.values()) + list(evacs.values()))
            cps[c] = cp

        def emit_evac(c):
            """SP: bounce slot of chunk c -> HBM, behind a nop carrying the
            copy-done wait."""
            spn = chain(nc.sync.nop(nofuse=True, hint=f"ev{c}"), "sp")
            dep(spn, cps[c], "copy done")
            ev = chain(
                nc.sync.dma_start(
                    out=ts_out[:, c * EV : (c + 1) * EV],
                    in_=evac_sb[:, c % EB, :],
                ),
                "sp",
            )
            dep(ev, spn, "after producer nop")
            demote(ev, [spn, cps[c]] + list(dmas.values()) + list(evacs.values()))
            evacs[c] = ev

        for c in range(nch):
            j = c % RING
            # Ring-slot WAR: carry waits on SP nops (the chain edges keep the
            # DMA behind them in the SP queue, so their hardware waits also
            # protect it).
            if c >= RING:
                n1 = chain(nc.sync.nop(nofuse=True, hint=f"war{c}a"), "sp")
                dep(n1, acts[c - RING], "act reader done")
                if (c - RING) in ttrs:
                    n1b = chain(nc.sync.nop(nofuse=True, hint=f"war{c}c"), "sp")
                    dep(n1b, ttrs[c - RING], "ttr reader done")
                n2 = chain(nc.sync.nop(nofuse=True, hint=f"war{c}b"), "sp")
                dep(n2, pe_last[c - RING], "pe reader done")
            if c - NPS in cps:
                # psum-bank WAR: gate this w DMA on the copy of chunk c-NPS;
                # mm[0]'s wait on the DMA sem then implies the bank is free.
                n3 = chain(nc.sync.nop(nofuse=True, hint=f"war{c}p"), "sp")
                dep(n3, cps[c - NPS], "psum bank free")
            dma = chain(
                nc.sync.dma_start(
                    out=w_ring[:, j, :, :, :], in_=w_in[:, c * CF : (c + 1) * CF]
                ),
                "sp",
            )
            # WAR waits live on the nops just above; DMA-vs-DMA WAW is ordered
            # by the HWDGE ring (FIFO per issuing engine).
            demote(dma, list(dmas.values()) + list(evacs.values()) + [dma_masks]
                   + [r for k in readers for r in readers[k]])
            dmas[c] = dma
            readers[c] = []

            # PE: selector sums.  One DoubleRow matmul per tile pair; member t
            # of pair p lands in psum row 2p+t.  mm[0] waits on the w DMA sem
            # directly; that also covers the masks DMA and the psum evac of
            # chunk c-NPS via the SP ring FIFO.
            pt = pst[c % NPS]
            # Deps provably covered elsewhere for every PE instruction of this
            # chunk: psum WAW vs chunk c-NPS rides the in-order PE queue; the
            # psum WAR vs evac_{c-NPS} rides the SP ring FIFO (mm[0]'s wait on
            # this chunk's w DMA implies the evac drained); the masks DMA
            # precedes every w DMA on the same ring.
            pe_covered = [dma_masks, cps.get(c - NPS), pe_last.get(c - 1),
                          pe_last.get(c - NPS), sel_last.get(c - NPS)]
            for p in range(PAIRS):
                mm = chain(
                    nc.tensor.matmul(
                        pt[:, 0:D],
                        lhsT=masks_sb[:, p, :, :],
                        rhs=w_ring[:, j, p, :, :],
                        start=(p == 0),
                        stop=(p == PAIRS - 1),
                        perf_mode=DR,
                    ),
                    "pe",
                )
                if p == 0:
                    demote(mm, pe_covered)
                else:
                    demote(mm, [dma] + pe_covered)
                readers[c].append(mm)
            sel_last[c] = mm
            # PE: gram squares for the last ng pairs (two half-feature
            # matmuls per pair).  One accumulation group spans the whole
            # kernel in its own psum bank; the diagonal is read once at the
            # end.
            for gi in range(ng):
                p = PAIRS - ng + gi
                for h in range(2):
                    gm = chain(
                        nc.tensor.matmul(
                            gps[:, :],
                            lhsT=w_ring[:, j, p, :, h * 64 : (h + 1) * 64],
                            rhs=w_ring[:, j, p, :, h * 64 : (h + 1) * 64],
                            start=(c == 0 and gi == 0 and h == 0),
                            stop=(c == nch - 1 and gi == ng - 1 and h == 1),
                            perf_mode=DR,
                            skip_group_check=True,
                        ),
                        "pe",
                    )
                    demote(gm, [dma, sel_last[c]] + pe_covered)
                    readers[c].append(gm)
            pe_last[c] = readers[c][-1]

            # ACT: squares of the first fdA columns, accumulated per chunk.
            act = chain(
                nc.scalar.activation(
                    scrA[:, c % 2, :],
                    w_ring[:, j, 0:na, :, :],
                    mybir.ActivationFunctionType.Square,
                    accum_out=sqa_cols[:, c : c + 1],
                ),
                "act",
            )
            demote(act, list(acts.values()))  # same-engine WAW on scrA
            acts[c] = act
            readers[c].append(act)

            # Lagged psum->SBUF copy, emitted BEFORE this chunk's ttr so the
            # SP-side waits on it (psum WAR, evac) never chain behind a fresh
            # ttr in the in-order DVE queue.
            if c >= CP_LAG:
                emit_cp(c - CP_LAG)

            # DVE: fused square+reduce of the middle fdV columns.
            if nv > 0:
                ttr = chain(
                    nc.vector.scalar_tensor_tensor(
                        out=scrV[:, c % 2, :],
                        in0=w_ring[:, j, na : na + nv, :, :],
                        scalar=1.0,
                        in1=w_ring[:, j, na : na + nv, :, :],
                        op0=mybir.AluOpType.mult,
                        op1=mybir.AluOpType.mult,
                        accum_out=sqv_cols[:, c : c + 1],
                    ),
                    "dve",
                )
                demote(ttr, list(ttrs.values()))  # scrV WAW same-engine
                ttrs[c] = ttr
                readers[c].append(ttr)

            # Lagged SBUF->HBM evac: by now the copy of chunk c-EV_LAG is long
            # done, so the SP queue doesn't stall and the prefetch stays deep.
            if c >= EV_LAG:
                emit_evac(c - EV_LAG)

        # Tail flush, same cadence as the main loop so bounce-slot reuse
        # (cp of chunk c vs evac DMA of chunk c-EB) stays correctly ordered.
        for cc in range(nch, nch + EV_LAG):
            if 0 <= cc - CP_LAG < nch and (cc - CP_LAG) not in cps:
                emit_cp(cc - CP_LAG)
            if 0 <= cc - EV_LAG < nch and (cc - EV_LAG) not in evacs:
                emit_evac(cc - EV_LAG)

        # Final gram readout: DVE copy psum -> SBUF once, then DMA out.
        gcp = chain(nc.vector.tensor_copy(gram_sb[:, :], gps[:, :]), "dve")
        dep(gcp, pe_last[nch - 1], "gram done")
        demote(gcp, [r for c2 in readers for r in readers[c2]
                     if r is not pe_last[nch - 1]]
               + list(cps.values()) + list(evacs.values()))

        # Outputs: each DMA waits on the last producer via an SP nop.
        outs = []
        for name, buf, src, last in (
            ("sqa", sqa_out, sqa_cols, acts[nch - 1]),
            ("sqv", sqv_out, sqv_cols, ttrs.get(nch - 1)),
            ("gram", gram_out, gram_sb, gcp),
        ):
            if last is None:
                continue
            spn = chain(nc.sync.nop(nofuse=True, hint=f"out_{name}"), "sp")
            dep(spn, last, f"{name} ready")
            od = chain(nc.sync.dma_start(out=buf[:, :], in_=src), "sp")
            dep(od, spn, "after producer nop")
            demote(od, [spn, last] + list(dmas.values()) + outs
                   + list(evacs.values()) + list(acts.values())
                   + list(ttrs.values()))
            outs.append(od)

        # Tail sync: cover every proc with single-wait SP nops.
        tails = [pe_last[nch - 1], acts[nch - 1]] + outs + [evacs[nch - 1]]
        if (nch - 1) in ttrs:
            tails.append(ttrs[nch - 1])
        for t in tails:
            nop = chain(nc.sync.nop(nofuse=True, hint="tailcover"), "sp")
            dep(nop, t, "tail")

    # The kernel-tail drain waits on every proc; its NOP struct cannot hold
    # that many sync waits and the SP-queue nops above already cover them.
    for blk in nc.m.functions[0].blocks:
        for inst in blk.instructions:
            if not isinstance(inst, mybir.InstDrain):
                continue
            si = inst.sync_info
            if si is None or len(si.on_wait) <= 2:
                continue
            inst.sync_info = mybir.SyncInfo(on_wait=[], on_update=list(si.on_update))

    return nc


def prepare_inputs(w1: np.ndarray, Y: np.ndarray, num_classes: int):
    """Class-sorted, per-class tile-padded, per-core partition-major fp8."""
    n = w1.shape[0]
    counts = np.bincount(Y, minlength=num_classes).astype(np.int64)
    tpc_class = (counts + P - 1) // P          # tiles per class
    pad_start = np.zeros(num_classes + 1, dtype=np.int64)
    np.cumsum(tpc_class, out=pad_start[1:])
    tt = int(pad_start[-1])                    # total real tiles
    t_core = -(-tt // N_CORES)                 # ceil
    t_core = -(-t_core // CHUNK) * CHUNK       # round up to chunk
    t_total = t_core * N_CORES

    order = np.argsort(Y, kind="stable")
    y_sorted = Y[order]
    class_start = np.zeros(num_classes, dtype=np.int64)
    class_start[1:] = np.cumsum(counts)[:-1]
    rank = np.arange(n, dtype=np.int64) - class_start[y_sorted]
    dest = pad_start[y_sorted] * P + rank

    w16 = np.zeros((t_total * P, D), dtype=NP_DT)
    w16[dest] = w1[order].astype(NP_DT)

    # selector masks: masks[k, p, t, m] = (m == 2p+t), laid out
    # [P, PAIRS*2*CHUNK]
    m1 = np.zeros((PAIRS, 2, CHUNK), dtype=NP_DT)
    for p in range(PAIRS):
        m1[p, 0, 2 * p] = 1
        m1[p, 1, 2 * p + 1] = 1
    masks = np.ascontiguousarray(
        np.broadcast_to(m1.reshape(1, PAIRS * 2 * CHUNK), (P, PAIRS * 2 * CHUNK))
    )
    in_maps = []
    for k in range(N_CORES):
        blk = w16[k * t_core * P : (k + 1) * t_core * P]
        wk = np.ascontiguousarray(
            blk.reshape(t_core, P, D).transpose(1, 0, 2).reshape(P, t_core * D)
        )
        in_maps.append({"w": wk, "masks": masks})
    return in_maps, t_core, pad_start, counts


def combine(results, t_core, pad_start, counts, n_total):
    """Host-side: tile sums -> class sums -> final scalar, in float64."""
    nch = t_core // CHUNK
    blocks = [r["ts_out"].astype(np.float64).reshape(CHUNK, nch, EV) for r in results]
    tile_sums = np.concatenate(
        [b[:, :, :D].transpose(1, 0, 2).reshape(t_core, D) for b in blocks],
        axis=0,
    )  # [t_total, D]
    num_classes = len(counts)
    totsq = 0.0
    for r in results:
        totsq += float(r["sqa_out"].astype(np.float64).sum())
        totsq += float(r["sqv_out"].astype(np.float64).sum())
        totsq += float(np.trace(r["gram_out"].astype(np.float64)))
    # per-class sums: classes are tile-aligned runs of tile_sums
    seg = np.add.reduceat(tile_sums[: pad_start[-1]], pad_start[:-1], axis=0) \
        if pad_start[-1] > 0 else np.zeros((num_classes, D))
    # reduceat quirk: empty segments (pad_start[c]==pad_start[c+1]) copy the
    # row at that index instead of 0 -- mask them out via counts.
    nz = counts > 0
    s = seg[nz]
    corr = float(((s * s).sum(axis=1) / counts[nz]).sum())
    return np.float32((totsq - corr) / n_total)


def run_sharded(w1: np.ndarray, Y: np.ndarray, num_classes: int, trace: bool = False):
    w1 = np.ascontiguousarray(np.asarray(w1, dtype=np.float32))
    Y = np.asarray(Y).astype(np.int64)
    in_maps, t_core, pad_start, counts = prepare_inputs(w1, Y, num_classes)
    nc = build_program(t_core)
    out = run_bass_kernel_spmd(nc, in_maps, list(range(N_CORES)), trace=trace)
    value = combine(out.results, t_core, pad_start, counts, w1.shape[0])
    return value, out


def kernel(w1, Y, num_classes=None):
    w1 = np.asarray(w1, dtype=np.float32)
    Y = np.asarray(Y)
    c = int(np.asarray(num_classes)) if num_classes is not None else 1000
    assert w1.ndim == 2 and w1.shape[1] == D
    value, _ = run_sharded(w1, Y, c, trace=False)
    return value


# revision 32
# speedup vs baseline: 1.4683x; 1.0141x over previous
"""Trainium2 Bass kernel for nn_DistributionLoss_6940667150680 (segment_reduce).

Math: with per-class sums S_c = sum_{i: Y_i=c} w_i and counts n_c,
    L2 = sum_i ||w_i - S_{Y_i}/n_{Y_i}||^2 = sum_i ||w_i||^2 - sum_c ||S_c||^2/n_c
so a single streaming pass over w1 suffices.

Sharding strategy (segment-key sharding): the host routes rows by class --
rows are stably sorted by label and each class is padded with zero rows to a
multiple of 128 so that every 128-row tile belongs to exactly one class.  The
padded tile stream is split evenly across the 8 cores.  Each core then only
needs per-TILE column sums (S_c = sum of its tiles' sums, reduced on host),
which turns the segment reduction into a dense streaming reduce.

v3 layout (per 64-tile chunk, CF = 64*128 = 8192 columns, fp8; the last
chunk may be shorter -- t_core is only rounded to a tile PAIR):
  - DMA: one 1 MB HBM->SBUF transfer per chunk (8 KB per-partition rows).
    The first two chunks are split into quarters/halves so compute starts
    as soon as the first 256 KB lands instead of after the full MB.
  - PE: one DoubleRow fp8 matmul per TILE PAIR: lhsT = [128, 2, 32]
    selector masks (member t of pair p%16 selects psum row 2(p%16)+t),
    rhs = [128, 2, 128] pair slab.  Pairs 0-15 accumulate into psum
    partitions 0-31, pairs 16-31 into partitions 32-63 (the 16 masks are
    shared between the groups) -> [64, 128] tile column sums per chunk.
  - PE (gram): the last NG pairs' squares also run on the PE: lhsT = rhs =
    [128, 2, 64] half-pair slab -> [64, 64] Gram accumulated in a dedicated
    psum bank across the WHOLE kernel; host reads only its diagonal
    (= per-feature sums of squares).
  - ACT: Square activation with accum_out on the first 256*NA columns.
  - DVE: fused square+reduce (scalar_tensor_tensor) on the next 256*NV cols.
  - Evac: DVE tensor_copy moves the [64, 128] psum tile sums to a bf16 SBUF
    bounce ring two chunks late (so DVE never waits on PE), and an SP-ring
    DMA ships it to HBM three chunks later still.  The psum WAR for bank
    reuse rides an SP nop ahead of each w DMA (mm[0]'s wait on the w DMA
    sem then implies the copy finished).
  - Host: fp8 cast + class-sorted layout (input prep), per-class reduction
    of tile sums, counts via bincount, final scalar in float64.
"""

import ml_dtypes
import numpy as np
from contextlib import ExitStack

import concourse.bass as bass
import concourse.tile as tile
from concourse import mybir
from concourse.bass_utils import run_bass_kernel_spmd

N_CORES = 8
D = 128           # feature dim
P = 128           # partitions / rows per tile
CHUNK = 64        # tiles per full DMA chunk
PAIRS = CHUNK // 2
NMASK = 16        # distinct selector masks (shared by the two 16-pair groups)
RING = 6          # w-ring depth in chunks
NPS = 6           # psum banks (round-robin per chunk)
CP_LAG = 2        # psum->SBUF copy issued this many chunks late (DVE no-stall)
EV_LAG = 5        # SBUF->HBM evac DMA issued this many chunks late
EB = 6            # evac SBUF bounce ring depth
NA = 12           # pairs squared on ACT  (256*NA columns)
NG = 10           # pairs squared on PE via Gram
# remaining PAIRS - NA - NG pairs squared on DVE
NP_DT = ml_dtypes.float8_e4m3
EV = D            # evac width per chunk: 128 tile-sum cols


def chunk_split(pairs_c):
    """(na, nv, ng) for a chunk with pairs_c pairs."""
    if pairs_c == PAIRS:
        return NA, PAIRS - NA - NG, NG
    na = max(1, round(NA * pairs_c / PAIRS))
    ng = round(NG * pairs_c / PAIRS)
    nv = pairs_c - na - ng
    if nv < 0:
        ng += nv
        nv = 0
    return na, nv, ng


def build_program(T: int):
    """Per-core program processing T tiles (T % 2 == 0)."""
    f32, f16 = mybir.dt.float32, mybir.dt.float16
    bf16 = mybir.dt.bfloat16
    fdt = mybir.dt.float8e4
    assert T % 2 == 0
    nch = -(-T // CHUNK)
    DR = mybir.MatmulPerfMode.DoubleRow
    cpairs = [PAIRS] * (nch - 1) + [(T - (nch - 1) * CHUNK) // 2]
    assert cpairs[-1] >= 1
    splits = [chunk_split(pc) for pc in cpairs]
    last_gram = max((c for c in range(nch) if splits[c][2] > 0), default=None)
    CF = CHUNK * D

    nc = bass.Bass()
    w_in = nc.dram_tensor("w", [P, T * D], fdt, kind="ExternalInput")
    masks_in = nc.dram_tensor("masks", [P, NMASK * 2 * 32], fdt, kind="ExternalInput")
    ts_out = nc.dram_tensor("ts_out", [32, nch * 2 * EV], bf16, kind="ExternalOutput")
    sqa_out = nc.dram_tensor("sqa_out", [P, max(nch, 1)], f32, kind="ExternalOutput")
    sqv_out = nc.dram_tensor("sqv_out", [P, max(nch, 1)], f32, kind="ExternalOutput")
    gram_out = nc.dram_tensor("gram_out", [64, 64], f32, kind="ExternalOutput")

    def dep(frm, to, why):
        tile.add_dep_helper(
            getattr(frm, "ins", frm), getattr(to, "ins", to), reason=why
        )

    def demote(inst, dep_insts):
        """Move provably-redundant sync deps to nosync (ordering only):
        same-engine WAW/WAR (in-order engines) and deps transitively covered
        by another emitted wait (ISA structs hold one sync wait each)."""
        inst = getattr(inst, "ins", inst)
        drop = set()
        for d in dep_insts:
            if d is None:
                continue
            drop.add(getattr(d, "ins", d).name)
        syncs = inst.take_sync_dependencies()
        nosyncs = inst.take_nosync_dependencies()
        for name in drop & set(syncs):
            syncs.discard(name)
            nosyncs.add(name)
        inst.set_sync_dependencies(syncs)
        inst.set_nosync_dependencies(nosyncs)

    # Pin each engine queue to emission order with demoted (nosync) chain
    # edges: the tile scheduler may otherwise reorder within a queue, which
    # breaks every "covered transitively via in-order engine" argument below.
    last_on = {}

    def chain(inst, engine):
        prev = last_on.get(engine)
        if prev is not None:
            dep(inst, prev, "queue order")
            demote(inst, [prev])
        last_on[engine] = inst
        return inst

    with tile.TileContext(nc) as tc, ExitStack() as ctx:
        const = ctx.enter_context(tc.tile_pool(name="const", bufs=1))
        psum = ctx.enter_context(tc.tile_pool(name="psum", bufs=1, space="PSUM"))

        masks_sb = const.tile([P, NMASK, 2, 32], fdt, name="masks_sb")
        w_ring = const.tile([P, RING, PAIRS, 2, D], fdt, name="w_ring")
        sqa_cols = const.tile([P, max(nch, 1)], f32, name="sqa_cols")
        sqv_cols = const.tile([P, max(nch, 1)], f32, name="sqv_cols")
        scrA = const.tile([P, 2, 256 * NA], f16, name="scrA")
        scrV = const.tile([P, 2, 256 * max(PAIRS - NA - NG, 1)], f16, name="scrV")
        evac_sb = const.tile([32, EB, 2 * EV], bf16, name="evac_sb")
        gram_sb = const.tile([64, 64], f32, name="gram_sb")

        # [32, 256]: the two 16-pair groups sit side by side in the free
        # dim (DR matmuls must target psum partition 0).
        pst = [psum.tile([32, 2 * EV], f32, name=f"pst{k}") for k in range(NPS)]
        # One gram bank accumulates squares across the whole kernel; its
        # diagonal is read once at the end.
        gps = psum.tile([64, 64], f32, name="gps")

        # Masks ride the ACT HWDGE ring so they don't delay the first w chunk
        # on the SP ring (ACT is idle until the first chunk lands anyway).
        dma_masks = chain(nc.scalar.dma_start(out=masks_sb, in_=masks_in[:, :]),
                          "act")

        dmas = {}      # chunk -> list of (pair_lo, pair_hi, dma_inst)
        readers = {}   # chunk -> instructions that read its ring slot
        acts = {}
        ttrs = {}
        pe_last = {}   # chunk -> last PE instruction of the chunk
        sel_last = {}
        cps = {}       # chunk -> DVE psum->SBUF copy
        evacs = {}     # chunk -> SBUF->HBM evac DMA

        def sec_dma(c, pair_lo):
            """The w-DMA section of chunk c whose LAST overlapping section
            covers pair_lo..; consumers keep only this section's dep (the SP
            ring FIFO implies all earlier sections of the chunk drained)."""
            for lo, hi, dd in dmas[c]:
                if lo <= pair_lo < hi:
                    return dd
            raise AssertionError((c, pair_lo))

        def all_dmas():
            return [dd for c2 in dmas for (_, _, dd) in dmas[c2]]

        def emit_cp(c):
            """DVE: psum bank of chunk c -> SBUF bounce slot c%EB (bf16).  A
            DVE nop carries the bounce-slot WAR (evac DMA of chunk c-EB
            drained); the copy itself carries the PE-done wait."""
            if c - EB in evacs:
                nd = chain(nc.vector.engine_nop(), "dve")
                dep(nd, evacs[c - EB], "bounce slot free")
            cp = chain(
                nc.vector.tensor_copy(evac_sb[:, c % EB, :], pst[c % NPS][:, :]),
                "dve",
            )
            dep(cp, pe_last[c], "psum ready")
            demote(cp, [r for c2 in readers for r in readers[c2]
                        if r is not pe_last[c]]
                   + list(cps.values()) + list(evacs.values()))
            cps[c] = cp

        def emit_evac(c):
            """SP: bounce slot of chunk c -> HBM, behind a nop carrying the
            copy-done wait."""
            spn = chain(nc.sync.nop(nofuse=True, hint=f"ev{c}"), "sp")
            dep(spn, cps[c], "copy done")
            ev = chain(
                nc.sync.dma_start(
                    out=ts_out[:, c * 2 * EV : (c + 1) * 2 * EV],
                    in_=evac_sb[:, c % EB, :],
                ),
                "sp",
            )
            dep(ev, spn, "after producer nop")
            demote(ev, [spn, cps[c]] + all_dmas() + list(evacs.values()))
            evacs[c] = ev

        for c in range(nch):
            j = c % RING
            pairs_c = cpairs[c]
            na, nv, ng = splits[c]
            # Ring-slot WAR: carry waits on SP nops (the chain edges keep the
            # DMA behind them in the SP queue, so their hardware waits also
            # protect it).
            if c >= RING:
                n1 = chain(nc.sync.nop(nofuse=True, hint=f"war{c}a"), "sp")
                dep(n1, acts[c - RING], "act reader done")
                if (c - RING) in ttrs:
                    n1b = chain(nc.sync.nop(nofuse=True, hint=f"war{c}c"), "sp")
                    dep(n1b, ttrs[c - RING], "ttr reader done")
                n2 = chain(nc.sync.nop(nofuse=True, hint=f"war{c}b"), "sp")
                dep(n2, pe_last[c - RING], "pe reader done")
            if c - NPS in cps:
                # psum-bank WAR: gate this w DMA on the copy of chunk c-NPS;
                # mm[0]'s wait on the DMA sem then implies the bank is free.
                n3 = chain(nc.sync.nop(nofuse=True, hint=f"war{c}p"), "sp")
                dep(n3, cps[c - NPS], "psum bank free")
            # w DMA, split into sections for the first chunks so compute can
            # start before the whole MB lands.
            nsec = 4 if c == 0 else (2 if c == 1 else 1)
            nsec = min(nsec, pairs_c)
            dmas[c] = []
            bounds = [pairs_c * s // nsec for s in range(nsec + 1)]
            prior_readers = [r for k in readers for r in readers[k]]
            for lo, hi in zip(bounds, bounds[1:]):
                dd = chain(
                    nc.sync.dma_start(
                        out=w_ring[:, j, lo:hi, :, :],
                        in_=w_in[:, c * CF + lo * 2 * D : c * CF + hi * 2 * D],
                    ),
                    "sp",
                )
                # WAR waits live on the nops just above; DMA-vs-DMA WAW is
                # ordered by the HWDGE ring (FIFO per issuing engine).
                demote(dd, all_dmas() + list(evacs.values()) + [dma_masks]
                       + prior_readers)
                dmas[c].append((lo, hi, dd))
            readers[c] = []

            # PE: selector sums.  One DoubleRow matmul per tile pair; member t
            # of pair p lands in psum row 32*(p//16) + 2*(p%16) + t.  Each
            # mm waits (at most) on its section's DMA sem; that also covers
            # the masks DMA (first LDW carries that wait separately) and the
            # psum evac of chunk c-NPS via the SP ring FIFO.
            pt = pst[c % NPS]
            pe_covered = [dma_masks, cps.get(c - NPS), pe_last.get(c - 1),
                          pe_last.get(c - NPS), sel_last.get(c - NPS)]
            kept = set()
            for p in range(pairs_c):
                g, q = divmod(p, NMASK)
                mm = chain(
                    nc.tensor.matmul(
                        pt[:, g * D : (g + 1) * D],
                        lhsT=masks_sb[:, q, :, :],
                        rhs=w_ring[:, j, p, :, :],
                        start=(p == 0),
                        stop=(p == pairs_c - 1),
                        perf_mode=DR,
                        skip_group_check=True,
                    ),
                    "pe",
                )
                sd = sec_dma(c, p)
                if sd.ins.name not in kept:
                    kept.add(sd.ins.name)
                    demote(mm, [d for d in all_dmas() if d is not sd]
                           + pe_covered)
                else:
                    demote(mm, all_dmas() + pe_covered)
                readers[c].append(mm)
            sel_last[c] = mm
            # PE: gram squares for the last ng pairs (two half-feature
            # matmuls per pair).  One accumulation group spans the whole
            # kernel in its own psum bank; the diagonal is read once at the
            # end.
            for gi in range(ng):
                p = pairs_c - ng + gi
                sd = sec_dma(c, p)
                for h in range(2):
                    gm = chain(
                        nc.tensor.matmul(
                            gps[:, :],
                            lhsT=w_ring[:, j, p, :, h * 64 : (h + 1) * 64],
                            rhs=w_ring[:, j, p, :, h * 64 : (h + 1) * 64],
                            start=(c == 0 and gi == 0 and h == 0),
                            stop=(c == last_gram and gi == ng - 1 and h == 1),
                            perf_mode=DR,
                            skip_group_check=True,
                        ),
                        "pe",
                    )
                    demote(gm, all_dmas() + [sel_last[c]] + pe_covered)
                    readers[c].append(gm)
            pe_last[c] = readers[c][-1]

            # ACT: squares of the first 256*na columns, accumulated per
            # chunk.  Keep only the dep on the LAST section it reads (ring
            # FIFO covers the earlier ones).
            act = chain(
                nc.scalar.activation(
                    scrA[:, c % 2, 0 : 256 * na],
                    w_ring[:, j, 0:na, :, :],
                    mybir.ActivationFunctionType.Square,
                    accum_out=sqa_cols[:, c : c + 1],
                ),
                "act",
            )
            sd = sec_dma(c, na - 1)
            demote(act, list(acts.values()) + [dma_masks]
                   + [d for d in all_dmas() if d is not sd])
            acts[c] = act
            readers[c].append(act)

            # Lagged psum->SBUF copy, emitted BEFORE this chunk's ttr so the
            # SP-side waits on it (psum WAR, evac) never chain behind a fresh
            # ttr in the in-order DVE queue.
            if c >= CP_LAG:
                emit_cp(c - CP_LAG)

            # DVE: fused square+reduce of the middle 256*nv columns.
            if nv > 0:
                ttr = chain(
                    nc.vector.scalar_tensor_tensor(
                        out=scrV[:, c % 2, 0 : 256 * nv],
                        in0=w_ring[:, j, na : na + nv, :, :],
                        scalar=1.0,
                        in1=w_ring[:, j, na : na + nv, :, :],
                        op0=mybir.AluOpType.mult,
                        op1=mybir.AluOpType.mult,
                        accum_out=sqv_cols[:, c : c + 1],
                    ),
                    "dve",
                )
                sd = sec_dma(c, na + nv - 1)
                demote(ttr, list(ttrs.values())
                       + [d for d in all_dmas() if d is not sd])
                ttrs[c] = ttr
                readers[c].append(ttr)

            # Lagged SBUF->HBM evac: by now the copy of chunk c-EV_LAG is long
            # done, so the SP queue doesn't stall and the prefetch stays deep.
            if c >= EV_LAG:
                emit_evac(c - EV_LAG)

        # Tail flush, same cadence as the main loop so bounce-slot reuse
        # (cp of chunk c vs evac DMA of chunk c-EB) stays correctly ordered.
        for cc in range(nch, nch + EV_LAG):
            if 0 <= cc - CP_LAG < nch and (cc - CP_LAG) not in cps:
                emit_cp(cc - CP_LAG)
            if 0 <= cc - EV_LAG < nch and (cc - EV_LAG) not in evacs:
                emit_evac(cc - EV_LAG)

        # Final gram readout: DVE copy psum -> SBUF once, then DMA out.
        gcp = chain(nc.vector.tensor_copy(gram_sb[:, :], gps[:, :]), "dve")
        glast = pe_last[last_gram] if last_gram is not None else pe_last[nch - 1]
        dep(gcp, glast, "gram done")
        demote(gcp, [r for c2 in readers for r in readers[c2] if r is not glast]
               + list(cps.values()) + list(evacs.values()))

        # Outputs: each DMA waits on the last producer via an SP nop.
        outs = []
        for name, buf, src, last in (
            ("sqa", sqa_out, sqa_cols, acts[nch - 1]),
            ("sqv", sqv_out, sqv_cols, ttrs.get(max(ttrs) if ttrs else 0)),
            ("gram", gram_out, gram_sb, gcp),
        ):
            if last is None:
                continue
            spn = chain(nc.sync.nop(nofuse=True, hint=f"out_{name}"), "sp")
            dep(spn, last, f"{name} ready")
            od = chain(nc.sync.dma_start(out=buf[:, :], in_=src), "sp")
            dep(od, spn, "after producer nop")
            demote(od, [spn, last] + all_dmas() + outs
                   + list(evacs.values()) + list(acts.values())
                   + list(ttrs.values()) + list(cps.values()))
            outs.append(od)

        # Tail sync: cover every proc with single-wait SP nops.
        tails = [pe_last[nch - 1], acts[nch - 1], gcp] + outs + [evacs[nch - 1]]
        if ttrs:
            tails.append(ttrs[max(ttrs)])
        for t in tails:
            nop = chain(nc.sync.nop(nofuse=True, hint="tailcover"), "sp")
            dep(nop, t, "tail")

    # The kernel-tail drain waits on every proc; its NOP struct cannot hold
    # that many sync waits and the SP-queue nops above already cover them.
    for blk in nc.m.functions[0].blocks:
        for inst in blk.instructions:
            if not isinstance(inst, mybir.InstDrain):
                continue
            si = inst.sync_info
            if si is None or len(si.on_wait) <= 2:
                continue
            inst.sync_info = mybir.SyncInfo(on_wait=[], on_update=list(si.on_update))

    return nc


def prepare_inputs(w1: np.ndarray, Y: np.ndarray, num_classes: int):
    """Class-sorted, per-class tile-padded, per-core partition-major fp8."""
    n = w1.shape[0]
    counts = np.bincount(Y, minlength=num_classes).astype(np.int64)
    tpc_class = (counts + P - 1) // P          # tiles per class
    pad_start = np.zeros(num_classes + 1, dtype=np.int64)
    np.cumsum(tpc_class, out=pad_start[1:])
    tt = int(pad_start[-1])                    # total real tiles
    t_core = -(-tt // N_CORES)                 # ceil
    t_core = -(-t_core // 2) * 2               # round up to a tile pair
    t_total = t_core * N_CORES

    order = np.argsort(Y, kind="stable")
    y_sorted = Y[order]
    class_start = np.zeros(num_classes, dtype=np.int64)
    class_start[1:] = np.cumsum(counts)[:-1]
    rank = np.arange(n, dtype=np.int64) - class_start[y_sorted]
    dest = pad_start[y_sorted] * P + rank

    w16 = np.zeros((t_total * P, D), dtype=NP_DT)
    w16[dest] = w1[order].astype(NP_DT)

    # selector masks: masks[k, q, t, m] = (m == 2q+t), laid out
    # [P, NMASK*2*32]
    m1 = np.zeros((NMASK, 2, 32), dtype=NP_DT)
    for q in range(NMASK):
        m1[q, 0, 2 * q] = 1
        m1[q, 1, 2 * q + 1] = 1
    masks = np.ascontiguousarray(
        np.broadcast_to(m1.reshape(1, NMASK * 2 * 32), (P, NMASK * 2 * 32))
    )
    in_maps = []
    for k in range(N_CORES):
        blk = w16[k * t_core * P : (k + 1) * t_core * P]
        wk = np.ascontiguousarray(
            blk.reshape(t_core, P, D).transpose(1, 0, 2).reshape(P, t_core * D)
        )
        in_maps.append({"w": wk, "masks": masks})
    return in_maps, t_core, pad_start, counts


def combine(results, t_core, pad_start, counts, n_total):
    """Host-side: tile sums -> class sums -> final scalar, in float64."""
    nch = -(-t_core // CHUNK)
    # ts_out rows: psum row 2q+t of group g in chunk c = tile 64c + 32g + 2q+t;
    # group g occupies cols [g*EV, (g+1)*EV) of chunk block c.
    tile_sums = np.concatenate(
        [
            r["ts_out"].astype(np.float64)
            .reshape(32, nch, 2, EV).transpose(1, 2, 0, 3).reshape(nch * CHUNK, EV)
            [:t_core]
            for r in results
        ],
        axis=0,
    )  # [8 * t_core, D] -- but each core block is t_core rows
    num_classes = len(counts)
    cpairs = [PAIRS] * (nch - 1) + [(t_core - (nch - 1) * CHUNK) // 2]
    splits = [chunk_split(pc) for pc in cpairs]
    va = [c for c in range(nch) if splits[c][0] > 0]
    vv = [c for c in range(nch) if splits[c][1] > 0]
    totsq = 0.0
    for r in results:
        totsq += float(r["sqa_out"].astype(np.float64)[:, va].sum())
        totsq += float(r["sqv_out"].astype(np.float64)[:, vv].sum())
        totsq += float(np.trace(r["gram_out"].astype(np.float64)))
    # per-class sums: classes are tile-aligned runs of tile_sums
    seg = np.add.reduceat(tile_sums[: pad_start[-1]], pad_start[:-1], axis=0) \
        if pad_start[-1] > 0 else np.zeros((num_classes, EV))
    # reduceat quirk: empty segments (pad_start[c]==pad_start[c+1]) copy the
    # row at that index instead of 0 -- mask them out via counts.
    nz = counts > 0
    s = seg[nz]
    corr = float(((s * s).sum(axis=1) / counts[nz]).sum())
    return np.float32((totsq - corr) / n_total)


def run_sharded(w1: np.ndarray, Y: np.ndarray, num_classes: int, trace: bool = False):
    w1 = np.ascontiguousarray(np.asarray(w1, dtype=np.float32))
    Y = np.asarray(Y).astype(np.int64)
    in_maps, t_core, pad_start, counts = prepare_inputs(w1, Y, num_classes)
    nc = build_program(t_core)
    out = run_bass_kernel_spmd(nc, in_maps, list(range(N_CORES)), trace=trace)
    value = combine(out.results, t_core, pad_start, counts, w1.shape[0])
    return value, out


def kernel(w1, Y, num_classes=None):
    w1 = np.asarray(w1, dtype=np.float32)
    Y = np.asarray(Y)
    c = int(np.asarray(num_classes)) if num_classes is not None else 1000
    assert w1.ndim == 2 and w1.shape[1] == D
    value, _ = run_sharded(w1, Y, c, trace=False)
    return value
